# revision 26
# baseline (speedup 1.0000x reference)
"""JIIF-style implicit-upsampling MLP on 8 Trainium2 NeuronCores.

Full inputs -> shard (batch, query-point) across 8 cores -> per-core Bass/Tile
kernel (gather + 5-layer MLP + softmax-combine) -> gather full output.

v2 design (NP = 32768 query points/core), all-bf16 matmul pipeline:
  * DRAM tables per core (bf16): tbl_fl [4097, 256] = concat(feat, lr_guide)
    per LR pixel + zero row (row 4096) for out-of-range zero padding;
    tbl_hr_lo/hi [32768, 128] = hr_guide per HR pixel split in two halves
    (dma_gather indices are int16; invalid hr points zeroed via mask mult).
  * On-device index math reproduces grid_sample_nearest bit-exactly
    (round-half-even via the 1.5*2^23 magic-add) in f32.
  * fl gathers use dma_gather(transpose=True): output lands directly as
    [128 ch, 2, 512 pts] column-major -> no PE transposes / evac copies.
  * hr gathers stay point-major (masks are point-major), 4 PE transposes.
  * The hr contribution to layer 0 (w0b @ hrT) is shift-invariant: computed
    once per 512-point tile into SBUF ("base") and added during L0
    evacuation (DVE add + ACT biased-relu), saving 24 matmuls/tile.
  * MLP (386-1024-512-256-128-2) as bf16 matmuls, N=512 points per tile,
    PSUM K-accumulation, fused bias+ReLU evacuation alternating DVE/ACT.
  * Final layer uses activations as the stationary operand to produce
    [pts, 2] directly in PSUM; softmax over the 4 shifts + weighted sum is
    done in point-major layout on DVE/ACT in f32.
"""
import sys

if "/opt/trn_rl_repo" not in sys.path:
    sys.path.insert(0, "/opt/trn_rl_repo")

import numpy as np
import ml_dtypes

import concourse.bass as bass
import concourse.bacc as bacc
import concourse.tile as tile
from concourse import mybir
from concourse.masks import make_identity

F32 = mybir.dt.float32
BF16 = mybir.dt.bfloat16
I16 = mybir.dt.int16
OP = mybir.AluOpType
ACTF = mybir.ActivationFunctionType
AX = mybir.AxisListType

MAGIC = 12582912.0  # 1.5 * 2**23 : forces round-to-nearest-even on f32 add

B, NFULL = 4, 65536
H_LR = 64
H_HR = 256
NCORES = 8
NP = (B * NFULL) // NCORES  # 32768 points per core
PIX_FL = H_LR * H_LR        # 4096
PIX_HR = H_HR * H_HR        # 65536
SHIFTS = [(-1.0 / 64, -1.0 / 64), (-1.0 / 64, 1.0 / 64),
          (1.0 / 64, -1.0 / 64), (1.0 / 64, 1.0 / 64)]


def build_program(npoints=NP):
    """Build the per-core Bass program. npoints must be a multiple of 512."""
    assert npoints % 512 == 0
    NQ = npoints // 128          # free-dim length of point-major tiles
    T = NQ // 4                  # number of 512-point tiles

    nc = bacc.Bacc("TRN2", target_bir_lowering=False, debug=False)

    tbl_fl = nc.dram_tensor("tbl_fl", [PIX_FL + 1, 256], BF16, kind="ExternalInput")
    tbl_hr_lo = nc.dram_tensor("tbl_hr_lo", [PIX_HR // 2, 128], BF16, kind="ExternalInput")
    tbl_hr_hi = nc.dram_tensor("tbl_hr_hi", [PIX_HR // 2, 128], BF16, kind="ExternalInput")
    wr_hlo_d = nc.dram_tensor("wr_hlo", [128, npoints // 16], I16, kind="ExternalInput")
    wr_hhi_d = nc.dram_tensor("wr_hhi", [128, npoints // 16], I16, kind="ExternalInput")
    wr_f_d = [nc.dram_tensor(f"wr_f{s}", [128, npoints // 16], I16,
                             kind="ExternalInput") for s in range(4)]
    mlo_d = nc.dram_tensor("mlo", [128, npoints // 128], BF16, kind="ExternalInput")
    mhi_d = nc.dram_tensor("mhi", [128, npoints // 128], BF16, kind="ExternalInput")
    rel_d = [nc.dram_tensor(f"rel{s}", [128, npoints // 128, 2], BF16,
                            kind="ExternalInput") for s in range(4)]
    w0a = nc.dram_tensor("w0a", [128, 1024], BF16, kind="ExternalInput")
    w0b = nc.dram_tensor("w0b", [128, 1024], BF16, kind="ExternalInput")
    w0c = nc.dram_tensor("w0c", [128, 1024], BF16, kind="ExternalInput")
    w0d = nc.dram_tensor("w0d", [2, 1024], BF16, kind="ExternalInput")
    w1 = nc.dram_tensor("w1", [128, 4096], BF16, kind="ExternalInput")
    w2 = nc.dram_tensor("w2", [128, 1024], BF16, kind="ExternalInput")
    w3 = nc.dram_tensor("w3", [128, 256], BF16, kind="ExternalInput")
    w4 = nc.dram_tensor("w4", [128, 2], BF16, kind="ExternalInput")
    bias0 = nc.dram_tensor("bias0", [128, 8], F32, kind="ExternalInput")
    bias1 = nc.dram_tensor("bias1", [128, 4], F32, kind="ExternalInput")
    bias2 = nc.dram_tensor("bias2", [128, 2], F32, kind="ExternalInput")
    bias3 = nc.dram_tensor("bias3", [128, 1], F32, kind="ExternalInput")
    bias4 = nc.dram_tensor("bias4", [128, 1], F32, kind="ExternalInput")
    out = nc.dram_tensor("out", [npoints], BF16, kind="ExternalOutput")

    evac_ctr = [0]

    def evac_relu(dst, src, bias_ap):
        # relu(src + bias), alternating DVE / ACT to balance engine load
        if evac_ctr[0] % 2 == 0:
            nc.vector.tensor_scalar(dst, src, bias_ap, 0.0, OP.add, OP.max)
        else:
            nc.scalar.activation(dst, src, ACTF.Relu, bias=bias_ap, scale=1.0)
        evac_ctr[0] += 1

    def evac_copy(dst, src):
        # copies go to ACT only: keeps the DVE FIFO short so the
        # PE-blocking t0 adds are reached with minimal queue latency
        nc.scalar.copy(dst, src)

    with tile.TileContext(nc) as tc:
        with tc.tile_pool(name="const", bufs=1) as cp, \
             tc.tile_pool(name="prol", bufs=1) as pp, \
             tc.tile_pool(name="gat", bufs=3) as gp, \
             tc.tile_pool(name="rhs", bufs=3) as rp, \
             tc.tile_pool(name="act", bufs=2) as ap, \
             tc.tile_pool(name="sm", bufs=2) as smp, \
             tc.tile_pool(name="ps", bufs=1, space="PSUM") as ps:

            ident = cp.tile([128, 128], BF16)
            make_identity(nc, ident[:])

            # ---- load weights / biases ----
            w0a_s = cp.tile([128, 1024], BF16)
            w0b_s = cp.tile([128, 1024], BF16)
            w0c_s = cp.tile([128, 1024], BF16)
            w0d_s = cp.tile([2, 1024], BF16)
            w1_s = cp.tile([128, 4096], BF16)
            w2_s = cp.tile([128, 1024], BF16)
            w3_s = cp.tile([128, 256], BF16)
            w4_s = cp.tile([128, 2], BF16)
            b0_s = cp.tile([128, 8], F32)
            b1_s = cp.tile([128, 4], F32)
            b2_s = cp.tile([128, 2], F32)
            b3_s = cp.tile([128, 1], F32)
            b4_s = cp.tile([128, 1], F32)
            for dst, src in [(w0a_s, w0a), (w0b_s, w0b), (w0c_s, w0c),
                             (w0d_s, w0d), (w1_s, w1), (w2_s, w2), (w3_s, w3),
                             (w4_s, w4), (b0_s, bias0), (b1_s, bias1),
                             (b2_s, bias2), (b3_s, bias3), (b4_s, bias4)]:
                nc.sync.dma_start(dst[:], src[:])

            # ---- load host-precomputed gather wraps / masks / rel ----
            wr_hlo = pp.tile([128, NQ * 8], I16)
            wr_hhi = pp.tile([128, NQ * 8], I16)
            nc.sync.dma_start(wr_hlo[:], wr_hlo_d[:])
            nc.scalar.dma_start(wr_hhi[:], wr_hhi_d[:])
            mlo_m = pp.tile([128, NQ], BF16)
            mhi_m = pp.tile([128, NQ], BF16)
            nc.sync.dma_start(mlo_m[:], mlo_d[:])
            nc.scalar.dma_start(mhi_m[:], mhi_d[:])
            idx_fl = []
            rel = []
            for s in range(4):
                eng = nc.sync if s % 2 == 0 else nc.scalar
                wf = pp.tile([128, NQ * 8], I16, tag=f"wf{s}")
                eng.dma_start(wf[:], wr_f_d[s][:])
                idx_fl.append(wf)
                rl = pp.tile([128, NQ, 2], BF16, tag=f"rel{s}")
                eng.dma_start(rl[:], rel_d[s][:])
                rel.append(rl)

            out_sb = pp.tile([128, NQ], BF16)

            # ---- main loop over 512-point tiles ----
            # Software-pipelined for the in-order engine queues:
            #  * each shift's L3/L4 tail is emitted AFTER the next shift's L0
            #    (psum-evac latency hides behind L0 compute);
            #  * each tile's softmax is emitted after the NEXT tile's first L1;
            #  * the hr gather->mask->transpose->evac chain for tile t+1 is
            #    emitted mid-tile-t so its DVE ops sit ahead of tile-t+1 evac
            #    backlog in the FIFO (and its gathers overlap tile-t compute).
            pend = [None]   # deferred L3/L4 of the previous shift
            smx = [None]    # deferred softmax of the previous tile

            def flush(slot):
                if slot[0] is not None:
                    slot[0]()
                    slot[0] = None

            def hr_prep(t):
                """Emit hr chain for tile t; returns its hrT tile."""
                q4 = slice(t * 4, t * 4 + 4)
                w32 = slice(t * 32, (t + 1) * 32)
                gh = gp.tile([128, 4, 128], BF16, tag="gh", name=f"gh_{t}")
                ghi = gp.tile([128, 4, 128], BF16, tag="ghi", name=f"ghi_{t}")
                nc.gpsimd.dma_gather(gh[:], tbl_hr_lo[:], wr_hlo[:, w32],
                                     num_idxs=512, num_idxs_reg=512,
                                     elem_size=128)
                nc.gpsimd.dma_gather(ghi[:], tbl_hr_hi[:], wr_hhi[:, w32],
                                     num_idxs=512, num_idxs_reg=512,
                                     elem_size=128)
                mlo_b = mlo_m[:, q4].unsqueeze(2).to_broadcast([128, 4, 128])
                mhi_b = mhi_m[:, q4].unsqueeze(2).to_broadcast([128, 4, 128])
                nc.vector.tensor_tensor(gh[:], gh[:], mlo_b, OP.mult)
                nc.vector.tensor_tensor(ghi[:], ghi[:], mhi_b, OP.mult)
                nc.vector.tensor_tensor(gh[:], gh[:], ghi[:], OP.add)
                pt_h = ps.tile([128, 512], BF16, tag="pth", bufs=1,
                               name=f"pth_{t}")
                for q in range(4):
                    nc.tensor.transpose(pt_h[:, q * 128:(q + 1) * 128],
                                        gh[:, q, :], ident[:])
                hrT = rp.tile([128, 512], BF16, tag="hrT", name=f"hrT_{t}")
                evac_copy(hrT[:], pt_h[:])
                # shift-invariant L0 part: base = w0b @ hrT
                base = ap.tile([128, 8, 512], BF16, tag="base",
                               name=f"base_{t}")
                for m in range(8):
                    pb = ps.tile([128, 512], F32, tag="pmm", bufs=5)
                    nc.tensor.matmul(pb[:], w0b_s[:, m * 128:(m + 1) * 128],
                                     hrT[:], start=True, stop=True)
                    evac_copy(base[:, m, :], pb[:])
                return hrT, base

            hrT_next = hr_prep(0)
            for t in range(T):
                q4 = slice(t * 4, t * 4 + 4)
                w32 = slice(t * 32, (t + 1) * 32)
                hrT, base = hrT_next
                p4 = ps.tile([128, 32], F32, tag="p4", bufs=1)

                for s in range(4):
                    gfl = gp.tile([128, 2, 512], BF16, tag="gfl", bufs=6)
                    nc.gpsimd.dma_gather(gfl[:], tbl_fl[:], idx_fl[s][:, w32],
                                         num_idxs=512, num_idxs_reg=512,
                                         elem_size=256, transpose=True)
                    featT = gfl[:, 0, :]
                    lrT = gfl[:, 1, :]

                    pt_r = ps.tile([2, 512], BF16, tag="ptr", bufs=1)
                    for q in range(4):
                        nc.tensor.transpose(pt_r[:, q * 128:(q + 1) * 128],
                                            rel[s][:, t * 4 + q, :], ident[:])
                    relT = rp.tile([2, 512], BF16, tag="relT")
                    evac_copy(relT[:], pt_r[:])

                    # L0: 386 -> 1024 (hr part comes from base)
                    a0 = ap.tile([128, 8, 512], BF16, tag="a0", bufs=3)
                    for m in range(8):
                        ms = slice(m * 128, (m + 1) * 128)
                        p0 = ps.tile([128, 512], F32, tag="pmm", bufs=5)
                        nc.tensor.matmul(p0[:], w0a_s[:, ms], featT,
                                         start=True, stop=False)
                        nc.tensor.matmul(p0[:], w0c_s[:, ms], lrT,
                                         start=False, stop=False)
                        nc.tensor.matmul(p0[:], w0d_s[:, ms], relT[:],
                                         start=False, stop=True)
                        t0 = ap.tile([128, 512], F32, tag="t0", bufs=4)
                        nc.vector.tensor_tensor(t0[:], p0[:], base[:, m, :],
                                                OP.add)
                        nc.scalar.activation(a0[:, m, :], t0[:], ACTF.Relu,
                                             bias=b0_s[:, m:m + 1], scale=1.0)

                    flush(pend)   # previous shift's (or tile's) L3/L4

                    # L1: 1024 -> 512
                    a1 = ap.tile([128, 4, 512], BF16, tag="a1")
                    for m in range(4):
                        p1 = ps.tile([128, 512], F32, tag="pmm", bufs=5)
                        for k in range(8):
                            nc.tensor.matmul(
                                p1[:],
                                w1_s[:, k * 512 + m * 128: k * 512 + (m + 1) * 128],
                                a0[:, k, :],
                                start=(k == 0), stop=(k == 7))
                        evac_relu(a1[:, m, :], p1[:], b1_s[:, m:m + 1])

                    if s == 0:
                        flush(smx)   # previous tile's softmax
                    if s == 2 and t + 1 < T:
                        hrT_next = hr_prep(t + 1)

                    # L2: 512 -> 256
                    a2 = ap.tile([128, 2, 512], BF16, tag="a2")
                    for m in range(2):
                        p2 = ps.tile([128, 512], F32, tag="pmm", bufs=5)
                        for k in range(4):
                            nc.tensor.matmul(
                                p2[:],
                                w2_s[:, k * 256 + m * 128: k * 256 + (m + 1) * 128],
                                a1[:, k, :],
                                start=(k == 0), stop=(k == 3))
                        evac_relu(a2[:, m, :], p2[:], b2_s[:, m:m + 1])

                    def make_shift_tail(s=s, a2=a2, p4=p4):
                        def f():
                            # L3: 256 -> 128
                            a3 = ap.tile([128, 512], BF16, tag="a3")
                            p3 = ps.tile([128, 512], F32, tag="pmm", bufs=5)
                            for k in range(2):
                                nc.tensor.matmul(p3[:],
                                                 w3_s[:, k * 128:(k + 1) * 128],
                                                 a2[:, k, :],
                                                 start=(k == 0), stop=(k == 1))
                            evac_relu(a3[:], p3[:], b3_s[:, 0:1])
                            # L4: 128 -> 2 -> [pts, 2] in PSUM
                            for q in range(4):
                                off = (q * 4 + s) * 2
                                nc.tensor.matmul(p4[:, off:off + 2],
                                                 a3[:, q * 128:(q + 1) * 128],
                                                 w4_s[:],
                                                 start=True, stop=True)
                        return f

                    pend[0] = make_shift_tail()

                    if s == 3:
                        def make_smx(p4=p4, q4=q4):
                            def f():
                                # softmax over shifts + weighted sum
                                p4c = smp.tile([128, 32], F32, tag="p4c")
                                nc.scalar.copy(p4c[:], p4[:])
                                p4v = p4c[:].rearrange("p (q s c) -> p q s c",
                                                       q=4, s=4)
                                mx = smp.tile([128, 4], F32, tag="mx")
                                nc.vector.tensor_reduce(mx[:], p4v[:, :, :, 1],
                                                        AX.X, OP.max)
                                e = smp.tile([128, 4, 4], F32, tag="e")
                                mxb = mx[:].unsqueeze(2).to_broadcast(
                                    [128, 4, 4])
                                nc.vector.tensor_tensor(e[:], p4v[:, :, :, 1],
                                                        mxb, OP.subtract)
                                nc.scalar.activation(e[:], e[:], ACTF.Exp)
                                ssum = smp.tile([128, 4], F32, tag="ssum")
                                nc.vector.tensor_reduce(ssum[:], e[:], AX.X,
                                                        OP.add)
                                nc.vector.tensor_tensor(e[:], e[:],
                                                        p4v[:, :, :, 0],
                                                        OP.mult)
                                num = smp.tile([128, 4], F32, tag="num")
                                nc.vector.tensor_reduce(num[:], e[:], AX.X,
                                                        OP.add)
                                rec = smp.tile([128, 4], F32, tag="rec")
                                nc.vector.reciprocal(rec[:], ssum[:])
                                nc.vector.tensor_tensor(num[:], num[:], rec[:],
                                                        OP.mult)
                                nc.vector.tensor_scalar(out_sb[:, q4], num[:],
                                                        b4_s[:, 0:1], None,
                                                        OP.add)
                            return f
                        smx[0] = make_smx()

            flush(pend)
            flush(smx)

            # transpose out_sb to HR-pixel-major via XBAR DMA (no PE/PSUM),
            # then one contiguous DMA out
            out_T2 = pp.tile([128, 2, 128], BF16)
            for c in range(2):
                nc.sync.dma_start_transpose(out_T2[:, c, :],
                                            out_sb[:, c * 128:(c + 1) * 128])
            nc.sync.dma_start(
                out[:].rearrange("(c q p) -> q c p", c=2, p=128), out_T2[:])

    nc.compile()
    return nc


def make_in_maps(feat, coord, hr_guide, lr_guide,
                 W0, b0, W1, b1, W2, b2, W3, b3, W4, b4,
                 npoints=NP, ncores=NCORES):
    """Host-side shard + repack. Returns per-core input dicts."""
    f32 = np.float32
    bf16 = ml_dtypes.bfloat16
    W0 = np.asarray(W0, f32)
    w0a = np.ascontiguousarray(W0[0:128]).astype(bf16)
    w0b = np.ascontiguousarray(W0[128:256] + W0[256:384]).astype(bf16)
    w0c = np.ascontiguousarray(-W0[256:384]).astype(bf16)
    w0d = np.ascontiguousarray(W0[384:386]).astype(bf16)
    w1r = np.ascontiguousarray(
        np.asarray(W1, f32).reshape(8, 128, 512).transpose(1, 0, 2).reshape(128, 4096)).astype(bf16)
    w2r = np.ascontiguousarray(
        np.asarray(W2, f32).reshape(4, 128, 256).transpose(1, 0, 2).reshape(128, 1024)).astype(bf16)
    w3r = np.ascontiguousarray(
        np.asarray(W3, f32).reshape(2, 128, 128).transpose(1, 0, 2).reshape(128, 256)).astype(bf16)
    w4r = np.ascontiguousarray(np.asarray(W4, f32)).astype(bf16)
    b0r = np.ascontiguousarray(np.asarray(b0, f32).reshape(8, 128).T)
    b1r = np.ascontiguousarray(np.asarray(b1, f32).reshape(4, 128).T)
    b2r = np.ascontiguousarray(np.asarray(b2, f32).reshape(2, 128).T)
    b3r = np.ascontiguousarray(np.asarray(b3, f32).reshape(1, 128).T)
    b4r = np.full((128, 1), np.asarray(b4, f32)[0], f32)

    per_batch = {}
    for b in range(B):
        fl = np.concatenate([
            np.asarray(feat[b], f32).reshape(128, PIX_FL).T,
            np.asarray(lr_guide[b], f32).reshape(128, PIX_FL).T], axis=1)
        tfl = np.zeros((PIX_FL + 1, 256), f32)
        tfl[:PIX_FL] = fl
        thr = np.asarray(hr_guide[b], f32).reshape(128, PIX_HR).T
        per_batch[b] = (np.ascontiguousarray(tfl).astype(bf16),
                        np.ascontiguousarray(thr[:PIX_HR // 2]).astype(bf16),
                        np.ascontiguousarray(thr[PIX_HR // 2:]).astype(bf16))

    halves = NFULL // npoints  # cores per batch
    in_maps = []
    for c in range(ncores):
        b = c // halves
        h = c % halves
        tfl, thr_lo, thr_hi = per_batch[b]
        cslice = np.asarray(coord[b, h * npoints:(h + 1) * npoints], f32)
        im = {
            "tbl_fl": tfl, "tbl_hr_lo": thr_lo, "tbl_hr_hi": thr_hi,
            "w0a": w0a, "w0b": w0b, "w0c": w0c, "w0d": w0d,
            "w1": w1r, "w2": w2r, "w3": w3r, "w4": w4r,
            "bias0": b0r, "bias1": b1r, "bias2": b2r, "bias3": b3r,
            "bias4": b4r,
        }
        im.update(host_index_products(cslice, npoints))
        in_maps.append(im)
    return in_maps


def host_index_products(coord_np, npoints):
    """Mirror of the original on-device index math (f32 elementwise, same op
    order, round-half-even) producing gather wraps, hr masks and rel coords."""
    f32 = np.float32
    bf16 = ml_dtypes.bfloat16
    NQ = npoints // 128
    c = coord_np.astype(f32)
    # I[p, q] = point n = q*128 + p
    cy = np.ascontiguousarray(c[:, 0].reshape(NQ, 128).T)
    cx = np.ascontiguousarray(c[:, 1].reshape(NQ, 128).T)

    def axis(cc, shift, H):
        src = (cc + f32(shift)).astype(f32) if shift is not None else cc
        v = ((src + f32(1.0)) * f32(H)).astype(f32)
        v = ((v - f32(1.0)) * f32(0.5)).astype(f32)
        r = np.rint(v).astype(f32)
        rc = np.clip(r, f32(0.0), f32(H - 1)).astype(f32)
        m = (r == rc).astype(f32)
        return rc, m

    def wrap16(idx_f32):
        ii = idx_f32.astype(np.int32).astype(np.int16)
        t = ii.reshape(8, 16, NQ)                       # [ph, r, q]
        w16 = t.transpose(1, 2, 0).reshape(16, NQ * 8)  # wr[r, q*8+ph]
        return np.ascontiguousarray(np.tile(w16, (8, 1)))

    out = {}
    ry_h, my_h = axis(cy, None, H_HR)
    rx_h, mx_h = axis(cx, None, H_HR)
    idx_hf = (ry_h * f32(H_HR) + rx_h).astype(f32)
    m_hr = (my_h * mx_h).astype(f32)
    hi = (idx_hf >= f32(PIX_HR // 2)).astype(f32)
    out["wr_hlo"] = wrap16(idx_hf * (f32(1.0) - hi))
    out["wr_hhi"] = wrap16((idx_hf - f32(PIX_HR // 2)) * hi)
    out["mlo"] = ((f32(1.0) - hi) * m_hr).astype(bf16)
    out["mhi"] = (hi * m_hr).astype(bf16)

    for s, (sy, sx) in enumerate(SHIFTS):
        ry, my = axis(cy, sy, H_LR)
        rx, mx = axis(cx, sx, H_LR)
        m = (my * mx).astype(f32)
        fidx = (ry * f32(H_LR) + rx).astype(f32)
        fidx = (fidx - f32(PIX_FL)) * m + f32(PIX_FL)
        out[f"wr_f{s}"] = wrap16(fidx)
        rl = np.empty((128, NQ, 2), bf16)
        for comp, (rcc, cc) in enumerate([(ry, cy), (rx, cx)]):
            qc = (rcc * f32(0.03125) + f32(-0.984375)).astype(f32)
            qc = (qc * m).astype(f32)
            qc = (cc - qc).astype(f32)
            rl[:, :, comp] = (qc * f32(64.0)).astype(f32).astype(bf16)
        out[f"rel{s}"] = rl
    return out


_CACHE = {}


def _get_program(npoints=NP):
    if npoints not in _CACHE:
        _CACHE[npoints] = build_program(npoints)
    return _CACHE[npoints]


def run_on_hw(inputs, trace=False):
    from concourse.bass_utils import run_bass_kernel_spmd
    nc = _get_program(NP)
    in_maps = make_in_maps(**inputs)
    res = run_bass_kernel_spmd(nc, in_maps, list(range(NCORES)), trace=trace)
    out = np.empty((B, NFULL, 1), np.float32)
    halves = NFULL // NP
    for c in range(NCORES):
        b, h = c // halves, c % halves
        out[b, h * NP:(h + 1) * NP, 0] = np.asarray(
            res.results[c]["out"]).astype(np.float32)
    return out, res


def kernel(**inputs):
    out, _ = run_on_hw(inputs, trace=False)
    return out


# revision 27
# speedup vs baseline: 1.1000x; 1.1000x over previous
"""JIIF-style implicit-upsampling MLP on 8 Trainium2 NeuronCores.

Full inputs -> shard (batch, query-point) across 8 cores -> per-core Bass/Tile
kernel (gather + 5-layer MLP + softmax-combine) -> gather full output.

v2 design (NP = 32768 query points/core), all-bf16 matmul pipeline:
  * DRAM tables per core (bf16): tbl_fl [4097, 256] = concat(feat, lr_guide)
    per LR pixel + zero row (row 4096) for out-of-range zero padding;
    tbl_hr_lo/hi [32768, 128] = hr_guide per HR pixel split in two halves
    (dma_gather indices are int16; invalid hr points zeroed via mask mult).
  * On-device index math reproduces grid_sample_nearest bit-exactly
    (round-half-even via the 1.5*2^23 magic-add) in f32.
  * fl gathers use dma_gather(transpose=True): output lands directly as
    [128 ch, 2, 512 pts] column-major -> no PE transposes / evac copies.
  * hr gathers stay point-major (masks are point-major), 4 PE transposes.
  * The hr contribution to layer 0 (w0b @ hrT) is shift-invariant: computed
    once per 512-point tile into SBUF ("base") and added during L0
    evacuation (DVE add + ACT biased-relu), saving 24 matmuls/tile.
  * MLP (386-1024-512-256-128-2) as bf16 matmuls, N=512 points per tile,
    PSUM K-accumulation, fused bias+ReLU evacuation alternating DVE/ACT.
  * Final layer uses activations as the stationary operand to produce
    [pts, 2] directly in PSUM; softmax over the 4 shifts + weighted sum is
    done in point-major layout on DVE/ACT in f32.
"""
import sys

if "/opt/trn_rl_repo" not in sys.path:
    sys.path.insert(0, "/opt/trn_rl_repo")

import numpy as np
import ml_dtypes

import concourse.bass as bass
import concourse.bacc as bacc
import concourse.tile as tile
from concourse import mybir
from concourse.masks import make_identity

F32 = mybir.dt.float32
BF16 = mybir.dt.bfloat16
I16 = mybir.dt.int16
OP = mybir.AluOpType
ACTF = mybir.ActivationFunctionType
AX = mybir.AxisListType

MAGIC = 12582912.0  # 1.5 * 2**23 : forces round-to-nearest-even on f32 add

B, NFULL = 4, 65536
H_LR = 64
H_HR = 256
NCORES = 8
NP = (B * NFULL) // NCORES  # 32768 points per core
PIX_FL = H_LR * H_LR        # 4096
PIX_HR = H_HR * H_HR        # 65536
SHIFTS = [(-1.0 / 64, -1.0 / 64), (-1.0 / 64, 1.0 / 64),
          (1.0 / 64, -1.0 / 64), (1.0 / 64, 1.0 / 64)]


def build_program(npoints=NP):
    """Build the per-core Bass program. npoints must be a multiple of 512."""
    assert npoints % 512 == 0
    NQ = npoints // 128          # free-dim length of point-major tiles
    T = NQ // 4                  # number of 512-point tiles

    nc = bacc.Bacc("TRN2", target_bir_lowering=False, debug=False)

    tbl_fl = nc.dram_tensor("tbl_fl", [PIX_FL + 1, 256], BF16, kind="ExternalInput")
    tbl_hr_lo = nc.dram_tensor("tbl_hr_lo", [PIX_HR // 2, 128], BF16, kind="ExternalInput")
    tbl_hr_hi = nc.dram_tensor("tbl_hr_hi", [PIX_HR // 2, 128], BF16, kind="ExternalInput")
    wr_hlo_d = nc.dram_tensor("wr_hlo", [128, npoints // 16], I16, kind="ExternalInput")
    wr_hhi_d = nc.dram_tensor("wr_hhi", [128, npoints // 16], I16, kind="ExternalInput")
    wr_f_d = [nc.dram_tensor(f"wr_f{s}", [128, npoints // 16], I16,
                             kind="ExternalInput") for s in range(4)]
    mlo_d = nc.dram_tensor("mlo", [128, npoints // 128], BF16, kind="ExternalInput")
    mhi_d = nc.dram_tensor("mhi", [128, npoints // 128], BF16, kind="ExternalInput")
    rel_d = [nc.dram_tensor(f"rel{s}", [128, npoints // 128, 2], BF16,
                            kind="ExternalInput") for s in range(4)]
    w0a = nc.dram_tensor("w0a", [128, 1024], BF16, kind="ExternalInput")
    w0b = nc.dram_tensor("w0b", [128, 1024], BF16, kind="ExternalInput")
    w0c = nc.dram_tensor("w0c", [128, 1024], BF16, kind="ExternalInput")
    w0d = nc.dram_tensor("w0d", [2, 1024], BF16, kind="ExternalInput")
    w1 = nc.dram_tensor("w1", [128, 4096], BF16, kind="ExternalInput")
    w2 = nc.dram_tensor("w2", [128, 1024], BF16, kind="ExternalInput")
    w3 = nc.dram_tensor("w3", [128, 256], BF16, kind="ExternalInput")
    w4 = nc.dram_tensor("w4", [128, 2], BF16, kind="ExternalInput")
    bias0 = nc.dram_tensor("bias0", [128, 8], F32, kind="ExternalInput")
    bias1 = nc.dram_tensor("bias1", [128, 4], F32, kind="ExternalInput")
    bias2 = nc.dram_tensor("bias2", [128, 2], F32, kind="ExternalInput")
    bias3 = nc.dram_tensor("bias3", [128, 1], F32, kind="ExternalInput")
    bias4 = nc.dram_tensor("bias4", [128, 1], F32, kind="ExternalInput")
    out = nc.dram_tensor("out", [npoints], BF16, kind="ExternalOutput")

    evac_ctr = [0]

    def evac_relu(dst, src, bias_ap):
        # relu(src + bias), alternating DVE / ACT to balance engine load
        if evac_ctr[0] % 2 == 0:
            nc.vector.tensor_scalar(dst, src, bias_ap, 0.0, OP.add, OP.max)
        else:
            nc.scalar.activation(dst, src, ACTF.Relu, bias=bias_ap, scale=1.0)
        evac_ctr[0] += 1

    def evac_copy(dst, src):
        if evac_ctr[0] % 2 == 0:
            nc.vector.tensor_copy(dst, src)
        else:
            nc.scalar.copy(dst, src)
        evac_ctr[0] += 1

    with tile.TileContext(nc) as tc:
        with tc.tile_pool(name="const", bufs=1) as cp, \
             tc.tile_pool(name="prol", bufs=1) as pp, \
             tc.tile_pool(name="gat", bufs=3) as gp, \
             tc.tile_pool(name="rhs", bufs=3) as rp, \
             tc.tile_pool(name="act", bufs=2) as ap, \
             tc.tile_pool(name="sm", bufs=2) as smp, \
             tc.tile_pool(name="ps", bufs=1, space="PSUM") as ps:

            ident = cp.tile([128, 128], BF16)
            make_identity(nc, ident[:])

            # ---- load weights / biases ----
            w0a_s = cp.tile([128, 1024], BF16)
            w0b_s = cp.tile([128, 1024], BF16)
            w0c_s = cp.tile([128, 1024], BF16)
            w0d_s = cp.tile([2, 1024], BF16)
            w1_s = cp.tile([128, 4096], BF16)
            w2_s = cp.tile([128, 1024], BF16)
            w3_s = cp.tile([128, 256], BF16)
            w4_s = cp.tile([128, 2], BF16)
            b0_s = cp.tile([128, 8], F32)
            b1_s = cp.tile([128, 4], F32)
            b2_s = cp.tile([128, 2], F32)
            b3_s = cp.tile([128, 1], F32)
            b4_s = cp.tile([128, 1], F32)
            for dst, src in [(w0a_s, w0a), (w0b_s, w0b), (w0c_s, w0c),
                             (w0d_s, w0d), (w1_s, w1), (w2_s, w2), (w3_s, w3),
                             (w4_s, w4), (b0_s, bias0), (b1_s, bias1),
                             (b2_s, bias2), (b3_s, bias3), (b4_s, bias4)]:
                nc.sync.dma_start(dst[:], src[:])

            # ---- load host-precomputed gather wraps / masks / rel ----
            wr_hlo = pp.tile([128, NQ * 8], I16)
            wr_hhi = pp.tile([128, NQ * 8], I16)
            nc.sync.dma_start(wr_hlo[:], wr_hlo_d[:])
            nc.scalar.dma_start(wr_hhi[:], wr_hhi_d[:])
            mlo_m = pp.tile([128, NQ], BF16)
            mhi_m = pp.tile([128, NQ], BF16)
            nc.sync.dma_start(mlo_m[:], mlo_d[:])
            nc.scalar.dma_start(mhi_m[:], mhi_d[:])
            idx_fl = []
            rel = []
            for s in range(4):
                eng = nc.sync if s % 2 == 0 else nc.scalar
                wf = pp.tile([128, NQ * 8], I16, tag=f"wf{s}")
                eng.dma_start(wf[:], wr_f_d[s][:])
                idx_fl.append(wf)
                rl = pp.tile([128, NQ, 2], BF16, tag=f"rel{s}")
                eng.dma_start(rl[:], rel_d[s][:])
                rel.append(rl)

            out_sb = pp.tile([128, NQ], BF16)

            # ---- main loop over 512-point tiles ----
            # Software-pipelined for the in-order engine queues:
            #  * each shift's L3/L4 tail is emitted AFTER the next shift's L0
            #    (psum-evac latency hides behind L0 compute);
            #  * each tile's softmax is emitted after the NEXT tile's first L1;
            #  * the hr gather->mask->transpose->evac chain for tile t+1 is
            #    emitted mid-tile-t so its DVE ops sit ahead of tile-t+1 evac
            #    backlog in the FIFO (and its gathers overlap tile-t compute).
            pend = [None]   # deferred L3/L4 of the previous shift
            smx = [None]    # deferred softmax of the previous tile

            def flush(slot):
                if slot[0] is not None:
                    slot[0]()
                    slot[0] = None

            def hr_prep(t):
                """Emit hr chain for tile t; returns its hrT tile."""
                q4 = slice(t * 4, t * 4 + 4)
                w32 = slice(t * 32, (t + 1) * 32)
                gh = gp.tile([128, 4, 128], BF16, tag="gh", name=f"gh_{t}")
                ghi = gp.tile([128, 4, 128], BF16, tag="ghi", name=f"ghi_{t}")
                nc.gpsimd.dma_gather(gh[:], tbl_hr_lo[:], wr_hlo[:, w32],
                                     num_idxs=512, num_idxs_reg=512,
                                     elem_size=128)
                nc.gpsimd.dma_gather(ghi[:], tbl_hr_hi[:], wr_hhi[:, w32],
                                     num_idxs=512, num_idxs_reg=512,
                                     elem_size=128)
                mlo_b = mlo_m[:, q4].unsqueeze(2).to_broadcast([128, 4, 128])
                mhi_b = mhi_m[:, q4].unsqueeze(2).to_broadcast([128, 4, 128])
                nc.vector.tensor_tensor(gh[:], gh[:], mlo_b, OP.mult)
                nc.vector.tensor_tensor(ghi[:], ghi[:], mhi_b, OP.mult)
                nc.vector.tensor_tensor(gh[:], gh[:], ghi[:], OP.add)
                pt_h = ps.tile([128, 512], BF16, tag="pth", bufs=1,
                               name=f"pth_{t}")
                for q in range(4):
                    nc.tensor.transpose(pt_h[:, q * 128:(q + 1) * 128],
                                        gh[:, q, :], ident[:])
                hrT = rp.tile([128, 512], BF16, tag="hrT", name=f"hrT_{t}")
                evac_copy(hrT[:], pt_h[:])
                # shift-invariant L0 part: base = w0b @ hrT
                base = ap.tile([128, 8, 512], BF16, tag="base",
                               name=f"base_{t}")
                for m in range(8):
                    pb = ps.tile([128, 512], F32, tag="pmm", bufs=5)
                    nc.tensor.matmul(pb[:], w0b_s[:, m * 128:(m + 1) * 128],
                                     hrT[:], start=True, stop=True)
                    evac_copy(base[:, m, :], pb[:])
                return hrT, base

            hrT_next = hr_prep(0)
            for t in range(T):
                q4 = slice(t * 4, t * 4 + 4)
                w32 = slice(t * 32, (t + 1) * 32)
                hrT, base = hrT_next
                p4 = ps.tile([128, 32], F32, tag="p4", bufs=1)

                for s in range(4):
                    gfl = gp.tile([128, 2, 512], BF16, tag="gfl", bufs=6)
                    nc.gpsimd.dma_gather(gfl[:], tbl_fl[:], idx_fl[s][:, w32],
                                         num_idxs=512, num_idxs_reg=512,
                                         elem_size=256, transpose=True)
                    featT = gfl[:, 0, :]
                    lrT = gfl[:, 1, :]

                    pt_r = ps.tile([2, 512], BF16, tag="ptr", bufs=1)
                    for q in range(4):
                        nc.tensor.transpose(pt_r[:, q * 128:(q + 1) * 128],
                                            rel[s][:, t * 4 + q, :], ident[:])
                    relT = rp.tile([2, 512], BF16, tag="relT")
                    evac_copy(relT[:], pt_r[:])

                    # L0: 386 -> 1024 (hr part comes from base)
                    a0 = ap.tile([128, 8, 512], BF16, tag="a0", bufs=3)
                    for m in range(8):
                        ms = slice(m * 128, (m + 1) * 128)
                        p0 = ps.tile([128, 512], F32, tag="pmm", bufs=5)
                        nc.tensor.matmul(p0[:], w0a_s[:, ms], featT,
                                         start=True, stop=False)
                        nc.tensor.matmul(p0[:], w0c_s[:, ms], lrT,
                                         start=False, stop=False)
                        nc.tensor.matmul(p0[:], w0d_s[:, ms], relT[:],
                                         start=False, stop=True)
                        t0 = ap.tile([128, 512], F32, tag="t0", bufs=4)
                        nc.vector.tensor_tensor(t0[:], p0[:], base[:, m, :],
                                                OP.add)
                        nc.scalar.activation(a0[:, m, :], t0[:], ACTF.Relu,
                                             bias=b0_s[:, m:m + 1], scale=1.0)

                    flush(pend)   # previous shift's (or tile's) L3/L4

                    # L1: 1024 -> 512
                    a1 = ap.tile([128, 4, 512], BF16, tag="a1")
                    for m in range(4):
                        p1 = ps.tile([128, 512], F32, tag="pmm", bufs=5)
                        for k in range(8):
                            nc.tensor.matmul(
                                p1[:],
                                w1_s[:, k * 512 + m * 128: k * 512 + (m + 1) * 128],
                                a0[:, k, :],
                                start=(k == 0), stop=(k == 7))
                        evac_relu(a1[:, m, :], p1[:], b1_s[:, m:m + 1])

                    if s == 0:
                        flush(smx)   # previous tile's softmax
                    if s == 2 and t + 1 < T:
                        hrT_next = hr_prep(t + 1)

                    # L2: 512 -> 256
                    a2 = ap.tile([128, 2, 512], BF16, tag="a2")
                    for m in range(2):
                        p2 = ps.tile([128, 512], F32, tag="pmm", bufs=5)
                        for k in range(4):
                            nc.tensor.matmul(
                                p2[:],
                                w2_s[:, k * 256 + m * 128: k * 256 + (m + 1) * 128],
                                a1[:, k, :],
                                start=(k == 0), stop=(k == 3))
                        evac_relu(a2[:, m, :], p2[:], b2_s[:, m:m + 1])

                    def make_shift_tail(s=s, a2=a2, p4=p4):
                        def f():
                            # L3: 256 -> 128
                            a3 = ap.tile([128, 512], BF16, tag="a3")
                            p3 = ps.tile([128, 512], F32, tag="pmm", bufs=5)
                            for k in range(2):
                                nc.tensor.matmul(p3[:],
                                                 w3_s[:, k * 128:(k + 1) * 128],
                                                 a2[:, k, :],
                                                 start=(k == 0), stop=(k == 1))
                            evac_relu(a3[:], p3[:], b3_s[:, 0:1])
                            # L4: 128 -> 2 -> [pts, 2] in PSUM
                            for q in range(4):
                                off = (q * 4 + s) * 2
                                nc.tensor.matmul(p4[:, off:off + 2],
                                                 a3[:, q * 128:(q + 1) * 128],
                                                 w4_s[:],
                                                 start=True, stop=True)
                        return f

                    pend[0] = make_shift_tail()

                    if s == 3:
                        def make_smx(p4=p4, q4=q4):
                            def f():
                                # softmax over shifts + weighted sum
                                p4c = smp.tile([128, 32], F32, tag="p4c")
                                nc.scalar.copy(p4c[:], p4[:])
                                p4v = p4c[:].rearrange("p (q s c) -> p q s c",
                                                       q=4, s=4)
                                mx = smp.tile([128, 4], F32, tag="mx")
                                nc.vector.tensor_reduce(mx[:], p4v[:, :, :, 1],
                                                        AX.X, OP.max)
                                e = smp.tile([128, 4, 4], F32, tag="e")
                                mxb = mx[:].unsqueeze(2).to_broadcast(
                                    [128, 4, 4])
                                nc.vector.tensor_tensor(e[:], p4v[:, :, :, 1],
                                                        mxb, OP.subtract)
                                nc.scalar.activation(e[:], e[:], ACTF.Exp)
                                ssum = smp.tile([128, 4], F32, tag="ssum")
                                nc.vector.tensor_reduce(ssum[:], e[:], AX.X,
                                                        OP.add)
                                nc.vector.tensor_tensor(e[:], e[:],
                                                        p4v[:, :, :, 0],
                                                        OP.mult)
                                num = smp.tile([128, 4], F32, tag="num")
                                nc.vector.tensor_reduce(num[:], e[:], AX.X,
                                                        OP.add)
                                rec = smp.tile([128, 4], F32, tag="rec")
                                nc.vector.reciprocal(rec[:], ssum[:])
                                nc.vector.tensor_tensor(num[:], num[:], rec[:],
                                                        OP.mult)
                                nc.vector.tensor_scalar(out_sb[:, q4], num[:],
                                                        b4_s[:, 0:1], None,
                                                        OP.add)
                            return f
                        smx[0] = make_smx()

            flush(pend)
            flush(smx)

            # transpose out_sb to HR-pixel-major via XBAR DMA (no PE/PSUM),
            # then one contiguous DMA out
            out_T2 = pp.tile([128, 2, 128], BF16)
            for c in range(2):
                nc.sync.dma_start_transpose(out_T2[:, c, :],
                                            out_sb[:, c * 128:(c + 1) * 128])
            nc.sync.dma_start(
                out[:].rearrange("(c q p) -> q c p", c=2, p=128), out_T2[:])

    nc.compile()
    return nc


def make_in_maps(feat, coord, hr_guide, lr_guide,
                 W0, b0, W1, b1, W2, b2, W3, b3, W4, b4,
                 npoints=NP, ncores=NCORES):
    """Host-side shard + repack. Returns per-core input dicts."""
    f32 = np.float32
    bf16 = ml_dtypes.bfloat16
    W0 = np.asarray(W0, f32)
    w0a = np.ascontiguousarray(W0[0:128]).astype(bf16)
    w0b = np.ascontiguousarray(W0[128:256] + W0[256:384]).astype(bf16)
    w0c = np.ascontiguousarray(-W0[256:384]).astype(bf16)
    w0d = np.ascontiguousarray(W0[384:386]).astype(bf16)
    w1r = np.ascontiguousarray(
        np.asarray(W1, f32).reshape(8, 128, 512).transpose(1, 0, 2).reshape(128, 4096)).astype(bf16)
    w2r = np.ascontiguousarray(
        np.asarray(W2, f32).reshape(4, 128, 256).transpose(1, 0, 2).reshape(128, 1024)).astype(bf16)
    w3r = np.ascontiguousarray(
        np.asarray(W3, f32).reshape(2, 128, 128).transpose(1, 0, 2).reshape(128, 256)).astype(bf16)
    w4r = np.ascontiguousarray(np.asarray(W4, f32)).astype(bf16)
    b0r = np.ascontiguousarray(np.asarray(b0, f32).reshape(8, 128).T)
    b1r = np.ascontiguousarray(np.asarray(b1, f32).reshape(4, 128).T)
    b2r = np.ascontiguousarray(np.asarray(b2, f32).reshape(2, 128).T)
    b3r = np.ascontiguousarray(np.asarray(b3, f32).reshape(1, 128).T)
    b4r = np.full((128, 1), np.asarray(b4, f32)[0], f32)

    per_batch = {}
    for b in range(B):
        fl = np.concatenate([
            np.asarray(feat[b], f32).reshape(128, PIX_FL).T,
            np.asarray(lr_guide[b], f32).reshape(128, PIX_FL).T], axis=1)
        tfl = np.zeros((PIX_FL + 1, 256), f32)
        tfl[:PIX_FL] = fl
        thr = np.asarray(hr_guide[b], f32).reshape(128, PIX_HR).T
        per_batch[b] = (np.ascontiguousarray(tfl).astype(bf16),
                        np.ascontiguousarray(thr[:PIX_HR // 2]).astype(bf16),
                        np.ascontiguousarray(thr[PIX_HR // 2:]).astype(bf16))

    halves = NFULL // npoints  # cores per batch
    in_maps = []
    for c in range(ncores):
        b = c // halves
        h = c % halves
        tfl, thr_lo, thr_hi = per_batch[b]
        cslice = np.asarray(coord[b, h * npoints:(h + 1) * npoints], f32)
        im = {
            "tbl_fl": tfl, "tbl_hr_lo": thr_lo, "tbl_hr_hi": thr_hi,
            "w0a": w0a, "w0b": w0b, "w0c": w0c, "w0d": w0d,
            "w1": w1r, "w2": w2r, "w3": w3r, "w4": w4r,
            "bias0": b0r, "bias1": b1r, "bias2": b2r, "bias3": b3r,
            "bias4": b4r,
        }
        im.update(host_index_products(cslice, npoints))
        in_maps.append(im)
    return in_maps


def host_index_products(coord_np, npoints):
    """Mirror of the original on-device index math (f32 elementwise, same op
    order, round-half-even) producing gather wraps, hr masks and rel coords."""
    f32 = np.float32
    bf16 = ml_dtypes.bfloat16
    NQ = npoints // 128
    c = coord_np.astype(f32)
    # I[p, q] = point n = q*128 + p
    cy = np.ascontiguousarray(c[:, 0].reshape(NQ, 128).T)
    cx = np.ascontiguousarray(c[:, 1].reshape(NQ, 128).T)

    def axis(cc, shift, H):
        src = (cc + f32(shift)).astype(f32) if shift is not None else cc
        v = ((src + f32(1.0)) * f32(H)).astype(f32)
        v = ((v - f32(1.0)) * f32(0.5)).astype(f32)
        r = np.rint(v).astype(f32)
        rc = np.clip(r, f32(0.0), f32(H - 1)).astype(f32)
        m = (r == rc).astype(f32)
        return rc, m

    def wrap16(idx_f32):
        ii = idx_f32.astype(np.int32).astype(np.int16)
        t = ii.reshape(8, 16, NQ)                       # [ph, r, q]
        w16 = t.transpose(1, 2, 0).reshape(16, NQ * 8)  # wr[r, q*8+ph]
        return np.ascontiguousarray(np.tile(w16, (8, 1)))

    out = {}
    ry_h, my_h = axis(cy, None, H_HR)
    rx_h, mx_h = axis(cx, None, H_HR)
    idx_hf = (ry_h * f32(H_HR) + rx_h).astype(f32)
    m_hr = (my_h * mx_h).astype(f32)
    hi = (idx_hf >= f32(PIX_HR // 2)).astype(f32)
    out["wr_hlo"] = wrap16(idx_hf * (f32(1.0) - hi))
    out["wr_hhi"] = wrap16((idx_hf - f32(PIX_HR // 2)) * hi)
    out["mlo"] = ((f32(1.0) - hi) * m_hr).astype(bf16)
    out["mhi"] = (hi * m_hr).astype(bf16)

    for s, (sy, sx) in enumerate(SHIFTS):
        ry, my = axis(cy, sy, H_LR)
        rx, mx = axis(cx, sx, H_LR)
        m = (my * mx).astype(f32)
        fidx = (ry * f32(H_LR) + rx).astype(f32)
        fidx = (fidx - f32(PIX_FL)) * m + f32(PIX_FL)
        out[f"wr_f{s}"] = wrap16(fidx)
        rl = np.empty((128, NQ, 2), bf16)
        for comp, (rcc, cc) in enumerate([(ry, cy), (rx, cx)]):
            qc = (rcc * f32(0.03125) + f32(-0.984375)).astype(f32)
            qc = (qc * m).astype(f32)
            qc = (cc - qc).astype(f32)
            rl[:, :, comp] = (qc * f32(64.0)).astype(f32).astype(bf16)
        out[f"rel{s}"] = rl
    return out


_CACHE = {}


def _get_program(npoints=NP):
    if npoints not in _CACHE:
        _CACHE[npoints] = build_program(npoints)
    return _CACHE[npoints]


def run_on_hw(inputs, trace=False):
    from concourse.bass_utils import run_bass_kernel_spmd
    nc = _get_program(NP)
    in_maps = make_in_maps(**inputs)
    res = run_bass_kernel_spmd(nc, in_maps, list(range(NCORES)), trace=trace)
    out = np.empty((B, NFULL, 1), np.float32)
    halves = NFULL // NP
    for c in range(NCORES):
        b, h = c // halves, c % halves
        out[b, h * NP:(h + 1) * NP, 0] = np.asarray(
            res.results[c]["out"]).astype(np.float32)
    return out, res


def kernel(**inputs):
    out, _ = run_on_hw(inputs, trace=False)
    return out
